# revision 31
# baseline (speedup 1.0000x reference)
"""2-layer GAT (nn_GAT_31490700214331) on 8 Trainium2 NeuronCores.

Strategy (dst-sharded, SPMD, per-core-rotated node layout), v2:
  - Nodes block-partitioned: core c owns nodes [c*6250, (c+1)*6250); every
    table on core c uses a rotated row order so local nodes are rows 0..6249
    and the SPMD program needs no core-dependent offsets.
  - Layer tables (t0: [h0|as0] f16 512B-pitch rows, t1: [h1|as1] f16
    1280B-pitch rows) are computed replicated on every core; per-dst-tile
    edges fetch source rows with dma_gather (int16 indices, lo/hi table
    split keeps indices < 32768).
  - alpha_dst is NOT gathered per edge: each core stashes its local tiles'
    ad columns in SBUF (adloc), builds a transposed incidence one-hot incT
    from a host-provided transposed dst-id row (PE ones-broadcast + DVE
    is_equal), and reads ad per edge-slot with tiny [128x128]@[128x8]
    matmuls. Denominators ride as 8 extra columns of the aggregation matmul.
  - Edge softmax is safe without segment-max (|e| small); aggregation is
    per-chunk incidence matmuls accumulating in PSUM.
  - Between layers the ELU'd hidden state is AllGather'd in fp8
    (6.25 MB instead of 12.5 MB), rotated into per-core order with
    partition-id-offset DMA reads in phase D.
  - alpha projections fold into the weight matmuls on the host:
    W0a=[256,16] / W1a=[128,16] give as/ad as extra psum columns.

Self-contained: call kernel(**inputs) with the full-problem arrays.
"""
import numpy as np
from contextlib import ExitStack

import concourse.bacc as bacc
import concourse.bass as bass
import concourse.mybir as mybir
from concourse.tile import TileContext
from concourse.bass_utils import run_bass_kernel_spmd

F16 = mybir.dt.float16
F32 = mybir.dt.float32
F8 = mybir.dt.float8e4
I16 = mybir.dt.int16

N = 50000
NFEAT = 256
NHID = 128
NCLASS = 64
HEADS = 8
SLOPE = 0.2
NCORES = 8
NLOC = N // NCORES           # 6250
LT = (NLOC + 127) // 128     # 49 local dst tiles
LAST_ROWS = NLOC - (LT - 1) * 128   # 106 rows in the last tile
GT = 392                     # global node tiles (392*128 = 50176)
GROWS = GT * 128
SPLIT = 25088                # low/high gather-table split (196 tiles)
SENT = 300.0                 # dst_rel sentinel for padding slots
T0W = 256                    # t0 row pitch (f16): [h0(128)|as0(8)|junk]
T1W = 640                    # t1 row pitch (f16): [h1(512)|as1(8)|junk]

_cache = {}


# --------------------------------------------------------------------------
# host-side preparation
# --------------------------------------------------------------------------

def _wrap_idx(idx):
    """[n] int -> [128, n//16] int16 wrapped gather-index layout."""
    n = idx.shape[0]
    assert n % 16 == 0
    w = idx.reshape(n // 16, 16).T.astype(np.int16)
    return np.tile(w, (8, 1))


def _prep_edges(src, dst):
    cores = []
    for c in range(NCORES):
        m = (dst >= c * NLOC) & (dst < (c + 1) * NLOC)
        s = src[m].astype(np.int64)
        d = dst[m].astype(np.int64) - c * NLOC
        order = np.argsort(d, kind="stable")
        s, d = s[order], d[order]
        s_rot = (s - c * NLOC) % N
        tiles = []
        for t in range(LT):
            sel = (d >= t * 128) & (d < (t + 1) * 128)
            st, dt = s_rot[sel], d[sel] - t * 128
            lo = st < SPLIT
            tiles.append((st[lo], dt[lo], st[~lo] - SPLIT, dt[~lo]))
        cores.append(tiles)
    nl = max(len(t[0]) for tl in cores for t in tl)
    nh = max(len(t[2]) for tl in cores for t in tl)
    NL = max(1, (nl + 127) // 128)
    NH = max(1, (nh + 127) // 128)
    assert NL * 128 <= 1024 and NH * 128 <= 1024, (NL, NH)
    CH = NL + NH

    out = []
    for c in range(NCORES):
        eil = np.zeros((LT, 128, NL * 8), np.int16)
        eih = np.zeros((LT, 128, NH * 8), np.int16)
        edr = np.full((LT, 128, CH), SENT, np.float16)
        edrT = np.full((LT, CH * 128), SENT, np.float16)
        for t in range(LT):
            sl, dl, sh, dh = cores[c][t]
            il = np.zeros(NL * 128, np.int64)
            il[: len(sl)] = sl
            ih = np.zeros(NH * 128, np.int64)
            ih[: len(sh)] = sh
            eil[t] = _wrap_idx(il)
            eih[t] = _wrap_idx(ih)
            rl = np.full(NL * 128, SENT)
            rl[: len(dl)] = dl
            rh = np.full(NH * 128, SENT)
            rh[: len(dh)] = dh
            flat = np.concatenate([rl, rh])
            edr[t] = flat.reshape(CH, 128).T.astype(np.float16)
            edrT[t] = flat.astype(np.float16)
        out.append(dict(eil=np.ascontiguousarray(eil),
                        eih=np.ascontiguousarray(eih),
                        edr=np.ascontiguousarray(edr),
                        edrT=np.ascontiguousarray(edrT)))
    return NL, NH, out


def _prep_inputs(x, edge_index, W0, a_src0, a_dst0, b0, W1, a_src1, a_dst1,
                 b1):
    src = np.asarray(edge_index[0]).astype(np.int64)
    dst = np.asarray(edge_index[1]).astype(np.int64)
    NL, NH, edata = _prep_edges(src, dst)
    CH = NL + NH

    def bd(a):  # [H, D] -> blockdiag [H*D, H]
        a = np.asarray(a, np.float32)
        H, D = a.shape
        m = np.zeros((H * D, H), np.float32)
        for h in range(H):
            m[h * D:(h + 1) * D, h] = a[h]
        return m

    W0 = np.asarray(W0, np.float32)
    W1 = np.asarray(W1, np.float32)
    W0a = np.concatenate([W0 @ bd(a_src0), W0 @ bd(a_dst0)], 1)  # [256, 16]
    # head-innermost feature interleave: new col d*8+h <- old col h*D+d
    perm0 = np.array([(f % 8) * 16 + f // 8 for f in range(128)])
    perm1 = np.array([(f % 8) * 64 + f // 8 for f in range(512)])
    W0cat = np.concatenate([W0[:, perm0], W0a], 1)               # [256, 144]
    W1a = np.concatenate([W1 @ bd(a_src1), W1 @ bd(a_dst1)], 1)  # [128, 16]

    x = np.asarray(x, np.float32)
    ident = np.eye(128, dtype=np.float16)
    # colio_rep[p, j*CH+c] = j ; rowio[p, :] = p
    colio_rep = np.repeat(np.arange(128, dtype=np.float16), CH)[None, :]
    colio_rep = np.ascontiguousarray(np.tile(colio_rep, (128, 1)))
    rowio = np.tile(np.arange(128, dtype=np.float16)[:, None], (1, CH * 128))
    b0b = np.tile(np.asarray(b0, np.float32)[None, :], (128, 1))
    b1b = np.tile(np.asarray(b1, np.float32)[None, :], (128, 1))

    in_maps = []
    for c in range(NCORES):
        rot = np.roll(np.arange(N), -c * NLOC)
        xr = np.zeros((GROWS, NFEAT), np.float16)
        xr[:N] = x[rot].astype(np.float16)
        xtt = xr.reshape(GROWS // 128, 128, 2, 128).transpose(0, 3, 2, 1)
        m = dict(
            xT=np.ascontiguousarray(xtt),
            W0=np.ascontiguousarray(
                W0cat.astype(np.float16).reshape(2, 128, NHID + 16)),
            W1=np.ascontiguousarray(W1[perm0][:, perm1].astype(np.float16)),
            W1a=np.ascontiguousarray(W1a[perm0].astype(np.float16)),
            b0b=np.ascontiguousarray(b0b[:, perm0]), b1b=b1b,
            ident=ident, colio_rep=colio_rep, rowio=np.ascontiguousarray(rowio),
            **edata[c],
        )
        in_maps.append(m)
    return NL, NH, in_maps


# --------------------------------------------------------------------------
# device program
# --------------------------------------------------------------------------

def build(NL, NH, lt=LT, gt=GT, phases="ABCDE"):
    CH = NL + NH
    NLI = NL * 128
    NHI = NH * 128

    nc = bacc.Bacc("TRN2")
    xT = nc.dram_tensor("xT", [GROWS // 128, 128, 2, 128], F16,
                        kind="ExternalInput")
    W0i = nc.dram_tensor("W0", [2, 128, NHID + 16], F16,
                         kind="ExternalInput")
    W1i = nc.dram_tensor("W1", [NHID, 512], F16, kind="ExternalInput")
    W1ai = nc.dram_tensor("W1a", [NHID, 16], F16, kind="ExternalInput")
    b0bi = nc.dram_tensor("b0b", [128, NHID], F32, kind="ExternalInput")
    b1bi = nc.dram_tensor("b1b", [128, NCLASS], F32, kind="ExternalInput")
    identi = nc.dram_tensor("ident", [128, 128], F16, kind="ExternalInput")
    colrepi = nc.dram_tensor("colio_rep", [128, 128 * CH], F16,
                             kind="ExternalInput")
    rowioi = nc.dram_tensor("rowio", [128, CH * 128], F16,
                            kind="ExternalInput")
    eili = nc.dram_tensor("eil", [lt, 128, NL * 8], I16,
                          kind="ExternalInput")
    eihi = nc.dram_tensor("eih", [lt, 128, NH * 8], I16,
                          kind="ExternalInput")
    edri = nc.dram_tensor("edr", [lt, 128, CH], F16, kind="ExternalInput")
    edrTi = nc.dram_tensor("edrT", [lt, CH * 128], F16,
                           kind="ExternalInput")
    out = nc.dram_tensor("out", [NLOC, NCLASS], F32, kind="ExternalOutput")

    with TileContext(nc) as tc, ExitStack() as stk:
        rg = {}
        for bs in {4, 2, 1, lt % 4 or 4, lt % 2 or 2}:
            rg[("lo", bs)] = nc.gpsimd.to_reg(bs * NLI)
            rg[("hi", bs)] = nc.gpsimd.to_reg(bs * NHI)
        dpool = stk.enter_context(
            tc.tile_pool(name="dram", bufs=1, space="DRAM"))
        t0lo = dpool.tile([SPLIT, T0W], F16, tag="t0lo")
        t0hi = dpool.tile([GROWS - SPLIT, T0W], F16, tag="t0hi")
        t1lo = dpool.tile([SPLIT, T1W], F16, tag="t1lo")
        t1hi = dpool.tile([GROWS - SPLIT, T1W], F16, tag="t1hi")
        CSP = 3072   # collective split column (end of B load-batch 2)
        agin_a = dpool.tile([128, CSP], F8, tag="agin_a")
        agin_b = dpool.tile([128, NLOC - CSP], F8, tag="agin_b")
        agout_a = dpool.tile([NCORES * 128, CSP], F8, tag="agout_a",
                             addr_space="Shared")
        agout_b = dpool.tile([NCORES * 128, NLOC - CSP], F8, tag="agout_b",
                             addr_space="Shared")

        cpool = stk.enter_context(tc.tile_pool(name="const", bufs=1))
        W0s = cpool.tile([128, 2, NHID + 16], F16)
        nc.sync.dma_start(out=W0s[:], in_=W0i.rearrange("k p n -> p k n"))
        W1s = cpool.tile([128, 512], F16)
        nc.sync.dma_start(out=W1s[:], in_=W1i[:])
        W1as = cpool.tile([128, 16], F16)
        nc.sync.dma_start(out=W1as[:], in_=W1ai[:])
        b0s = cpool.tile([128, NHID], F32)
        nc.sync.dma_start(out=b0s[:], in_=b0bi[:])
        b1s = cpool.tile([128, NCLASS], F32)
        nc.sync.dma_start(out=b1s[:], in_=b1bi[:])
        idents = cpool.tile([128, 128], F16)
        nc.sync.dma_start(out=idents[:], in_=identi[:])
        colreps = cpool.tile([128, 128, CH], F16)
        nc.sync.dma_start(out=colreps[:],
                          in_=colrepi.rearrange("p (j c) -> p j c", c=CH))
        rowios = cpool.tile([128, CH * 128], F16)
        nc.sync.dma_start(out=rowios[:], in_=rowioi[:])
        ones1 = cpool.tile([1, 128], F16)
        nc.vector.memset(ones1[:], 1.0)
        adloc0 = cpool.tile([128, lt * 8], F16)
        adloc1 = cpool.tile([128, lt * 8], F16)

        def tbl_write(eng, tlo, thi, g0, row_ap, w, nt):
            """write nt row-tiles (128 rows each) starting at table row g0"""
            rows = nt * 128
            if g0 + rows <= SPLIT:
                eng.dma_start(
                    out=tlo[g0:g0 + rows, 0:w]
                    .rearrange("(g p) w -> p g w", p=128),
                    in_=row_ap)
            elif g0 >= SPLIT:
                o = g0 - SPLIT
                eng.dma_start(
                    out=thi[o:o + rows, 0:w]
                    .rearrange("(g p) w -> p g w", p=128),
                    in_=row_ap)
            else:
                k = (SPLIT - g0) // 128
                eng.dma_start(
                    out=tlo[g0:SPLIT, 0:w]
                    .rearrange("(g p) w -> p g w", p=128),
                    in_=row_ap[:, 0:k, :])
                eng.dma_start(
                    out=thi[0:rows - k * 128, 0:w]
                    .rearrange("(g p) w -> p g w", p=128),
                    in_=row_ap[:, k:nt, :])

        # ---------------- phase A: layer-0 tables (replicated) ------------
        if "A" in phases:
            AB = 14
            with ExitStack() as pa:
                xp = pa.enter_context(tc.tile_pool(name="pa_x", bufs=3))
                pp = pa.enter_context(
                    tc.tile_pool(name="pa_ps", bufs=1, space="PSUM"))
                rp = pa.enter_context(tc.tile_pool(name="pa_row", bufs=3))
                assert gt % AB == 0
                for i in range(gt // AB):
                    xa = xp.tile([128, AB, 2, 128], F16, tag="xa")
                    nc.sync.dma_start(
                        out=xa[:],
                        in_=xT[AB * i:AB * (i + 1)].rearrange(
                            "g p k j -> p g k j"))
                    row = rp.tile([128, AB, 136], F16, tag="row")
                    for g2 in range(AB):
                        ps = pp.tile([128, NHID + 16], F32, tag=f"ps{g2 % 4}")
                        nc.tensor.matmul(ps[:], xa[:, g2, 0, :], W0s[:, 0, :],
                                         start=True, stop=False)
                        nc.tensor.matmul(ps[:], xa[:, g2, 1, :], W0s[:, 1, :],
                                         start=False, stop=True)
                        nc.vector.tensor_copy(row[:, g2, :], ps[:, 0:136])
                        g = AB * i + g2
                        if g < lt:
                            nc.scalar.copy(adloc0[:, g * 8:(g + 1) * 8],
                                           ps[:, 136:144])
                    eng = nc.scalar if i % 2 else nc.sync
                    tbl_write(eng, t0lo, t0hi, i * AB * 128, row[:], 136, AB)

        # ---------------- shared edge phase -------------------------------
        def edge_phase(layer, tlo, thi, adloc, fdim, trow, GB,
                       post_fn, hook=None):
            D = fdim // HEADS
            LB = 8   # load batch (tiles)
            with ExitStack() as pb:
                ii = pb.enter_context(tc.tile_pool(name=f"ix{layer}", bufs=3))
                i2 = pb.enter_context(tc.tile_pool(name=f"i2{layer}", bufs=1))
                gp = pb.enter_context(tc.tile_pool(
                    name=f"gg{layer}", bufs=3 if layer else 4))
                rp2 = pb.enter_context(
                    tc.tile_pool(name=f"rh{layer}", bufs=3))
                rp3 = pb.enter_context(
                    tc.tile_pool(name=f"rr{layer}", bufs=3 if layer else 4))
                pp2 = pb.enter_context(
                    tc.tile_pool(name=f"ps{layer}", bufs=2, space="PSUM"))
                pp1b = pb.enter_context(
                    tc.tile_pool(name=f"p1{layer}", bufs=1, space="PSUM"))
                op = pb.enter_context(tc.tile_pool(name=f"po{layer}", bufs=4))

                for l0 in range(0, lt, LB):
                    lb = min(LB, lt - l0)
                    il_s = ii.tile([128, LB, NL * 8], I16, tag="il")
                    nc.sync.dma_start(
                        out=il_s[:, 0:lb, :],
                        in_=eili[l0:l0 + lb].rearrange("t p w -> p t w"))
                    ih_s = ii.tile([128, LB, NH * 8], I16, tag="ih")
                    nc.sync.dma_start(
                        out=ih_s[:, 0:lb, :],
                        in_=eihi[l0:l0 + lb].rearrange("t p w -> p t w"))
                    drs = ii.tile([128, LB, CH], F16, tag="dr")
                    nc.scalar.dma_start(
                        out=drs[:, 0:lb, :],
                        in_=edri[l0:l0 + lb].rearrange("t p w -> p t w"))
                    ag_stage = None
                    if layer == 0:
                        ag_stage = op.tile([128, LB, 128], F8, tag="ag")

                    drts = i2.tile([1, LB, CH * 128], F16, tag="drt")
                    nc.scalar.dma_start(
                        out=drts[0:1, 0:lb, :], in_=edrTi[l0:l0 + lb])
                    for s0 in range(0, lb, GB):
                        gb = min(GB, lb - s0)
                        Glo = gp.tile([128, GB * NL, trow], F16, tag="glo")
                        nc.gpsimd.dma_gather(
                            Glo[:, 0:gb * NL, :], tlo[:],
                            il_s[:, s0:s0 + gb, :]
                            .rearrange("p t w -> p (t w)"),
                            gb * NLI, rg[("lo", gb)], trow)
                        Ghi = gp.tile([128, GB * NH, trow], F16, tag="ghi")
                        nc.gpsimd.dma_gather(
                            Ghi[:, 0:gb * NH, :], thi[:],
                            ih_s[:, s0:s0 + gb, :]
                            .rearrange("p t w -> p (t w)"),
                            gb * NHI, rg[("hi", gb)], trow)

                        for u in range(gb):
                            tt = s0 + u           # index within load batch
                            t = l0 + tt           # tile index
                            gl = Glo[:, u * NL:(u + 1) * NL, :]
                            gh = Ghi[:, u * NH:(u + 1) * NH, :]

                            # transposed incidence one-hot via ones-broadcast
                            drB16 = rp2.tile([128, CH * 128], F16, tag="drB")
                            quart = (CH * 128) // 4
                            assert quart <= 512
                            for hh in range(4):
                                drB = pp1b.tile([128, quart], F32,
                                                tag=f"drB{hh % 2}")
                                nc.tensor.matmul(
                                    drB[:], ones1[:],
                                    drts[0:1, tt,
                                         hh * quart:(hh + 1) * quart],
                                    start=True, stop=True)
                                nc.scalar.copy(
                                    drB16[:, hh * quart:(hh + 1) * quart],
                                    drB[:])
                            incT = rp2.tile([128, CH * 128], F16, tag="incT")
                            nc.vector.tensor_tensor(
                                out=incT[:], in0=drB16[:], in1=rowios[:],
                                op=mybir.AluOpType.is_equal)
                            # ad per edge slot: [K=128d, M=128slot]^T @ [K,8]
                            adsp = pp1b.tile([128, CH, 8], F32,
                                             tag=f"adsp{t % 2}")
                            for ch in range(CH):
                                nc.tensor.matmul(
                                    adsp[:, ch, :],
                                    incT[:, ch * 128:(ch + 1) * 128],
                                    adloc[:, t * 8:(t + 1) * 8],
                                    start=True, stop=True)

                            # incidence (j-major for 2x DVE)
                            inc = rp3.tile([128, 128, CH], F16, tag="inc")
                            nc.vector.tensor_tensor(
                                out=inc[:],
                                in0=drs[:, tt, :].unsqueeze(1)
                                .broadcast_to([128, 128, CH]),
                                in1=colreps[:],
                                op=mybir.AluOpType.is_equal)

                            # EX = exp(prelu(as + ad)) into R[:, :, fdim:]
                            R = rp3.tile([128, CH, fdim + 8], F16, tag="R")
                            nc.vector.tensor_tensor(
                                out=R[:, 0:NL, fdim:fdim + 8],
                                in0=gl[:, :, fdim:fdim + 8],
                                in1=adsp[:, 0:NL, :],
                                op=mybir.AluOpType.add)
                            nc.vector.tensor_tensor(
                                out=R[:, NL:CH, fdim:fdim + 8],
                                in0=gh[:, :, fdim:fdim + 8],
                                in1=adsp[:, NL:CH, :],
                                op=mybir.AluOpType.add)
                            ex = R[:, :, fdim:fdim + 8]
                            # prelu(e) = e + (1-slope)*relu(-e)
                            exn = rp3.tile([128, CH, 8], F16, tag="exn")
                            nc.scalar.activation(
                                exn[:], ex,
                                mybir.ActivationFunctionType.Relu,
                                scale=-(1.0 - SLOPE))
                            nc.vector.tensor_tensor(
                                out=ex, in0=ex, in1=exn[:],
                                op=mybir.AluOpType.add)
                            nc.scalar.activation(
                                ex, ex, mybir.ActivationFunctionType.Exp)

                            # R = h * EX (head-innermost broadcast)
                            nc.vector.tensor_tensor(
                                out=R[:, 0:NL, 0:fdim]
                                .rearrange("p c (d h) -> p c d h", h=HEADS),
                                in0=gl[:, :, 0:fdim]
                                .rearrange("p c (d h) -> p c d h", h=HEADS),
                                in1=R[:, 0:NL, fdim:fdim + 8].unsqueeze(2)
                                .broadcast_to([128, NL, D, HEADS]),
                                op=mybir.AluOpType.mult)
                            nc.vector.tensor_tensor(
                                out=R[:, NL:CH, 0:fdim]
                                .rearrange("p c (d h) -> p c d h", h=HEADS),
                                in0=gh[:, :, 0:fdim]
                                .rearrange("p c (d h) -> p c d h", h=HEADS),
                                in1=R[:, NL:CH, fdim:fdim + 8].unsqueeze(2)
                                .broadcast_to([128, NH, D, HEADS]),
                                op=mybir.AluOpType.mult)

                            # aggregate (+denominators merged when they fit
                            # a single PSUM bank)
                            if fdim + 8 <= 512:
                                P = pp2.tile([128, fdim + 8], F32, tag="P1")
                                for ch in range(CH):
                                    nc.tensor.matmul(
                                        P[:], inc[:, :, ch], R[:, ch, :],
                                        start=(ch == 0), stop=(ch == CH - 1))
                                P1, P2 = P[:, 0:fdim], P[:, fdim:fdim + 8]
                            else:
                                P1t = pp2.tile([128, fdim], F32, tag="P1")
                                for ch in range(CH):
                                    nc.tensor.matmul(
                                        P1t[:], inc[:, :, ch],
                                        R[:, ch, 0:fdim],
                                        start=(ch == 0), stop=(ch == CH - 1))
                                P2t = pp2.tile([128, 8], F32, tag="P2")
                                for ch in range(CH):
                                    nc.tensor.matmul(
                                        P2t[:], inc[:, :, ch],
                                        R[:, ch, fdim:fdim + 8],
                                        start=(ch == 0), stop=(ch == CH - 1))
                                P1, P2 = P1t[:], P2t[:]
                            post_fn(t, tt, P1, P2, op, pp2, ag_stage)

                    if layer == 0:
                        c0 = l0 * 128
                        cn = min(lb * 128, NLOC - c0)
                        if c0 < 3072:
                            dst = agin_a[:, c0:c0 + cn]
                        else:
                            dst = agin_b[:, c0 - 3072:c0 - 3072 + cn]
                        nc.scalar.dma_start(
                            out=dst,
                            in_=ag_stage[:]
                            .rearrange("p t j -> p (t j)")[:, 0:cn])
                    if hook is not None:
                        hook(l0)

        # ---- L0 post: softmax-div, +b0, ELU, transpose, stage fp8 --------
        def post0(t, tt, P1, P2, op, pp2, ag_stage):
            r8 = op.tile([128, 8], F32, tag="r8")
            nc.vector.tensor_scalar_add(r8[:], P2, 1e-16)
            nc.vector.reciprocal(r8[:], r8[:])
            z = op.tile([128, NHID], F32, tag="z")
            nc.vector.tensor_tensor(
                out=z[:].rearrange("p (d h) -> p d h", h=HEADS),
                in0=P1.rearrange("p (d h) -> p d h", h=HEADS),
                in1=r8[:].unsqueeze(1).broadcast_to([128, 16, HEADS]),
                op=mybir.AluOpType.mult)
            nc.vector.tensor_tensor(out=z[:], in0=z[:], in1=b0s[:],
                                    op=mybir.AluOpType.add)
            zm = op.tile([128, NHID], F16, tag="zm")
            nc.scalar.activation(zm[:], z[:],
                                 mybir.ActivationFunctionType.Relu,
                                 scale=-1.0)
            nc.scalar.activation(zm[:], zm[:],
                                 mybir.ActivationFunctionType.Exp,
                                 scale=-1.0)
            zp = op.tile([128, NHID], F16, tag="zp")
            nc.scalar.activation(zp[:], z[:],
                                 mybir.ActivationFunctionType.Relu)
            h1 = op.tile([128, NHID], F16, tag="h1")
            nc.vector.tensor_tensor(out=h1[:], in0=zp[:], in1=zm[:],
                                    op=mybir.AluOpType.add)
            nc.vector.tensor_scalar_add(h1[:], h1[:], -1.0)
            pst = pp2.tile([128, 128], F16, tag="pst")
            nc.tensor.transpose(pst[:], h1[:], idents[:])
            nc.vector.tensor_copy(ag_stage[:, tt, :], pst[:])

        if "B" in phases:
            def b_hook(l0):
                if l0 == 16 and "C" in phases:
                    nc.gpsimd.collective_compute(
                        "AllGather", mybir.AluOpType.bypass,
                        replica_groups=[list(range(NCORES))],
                        ins=[agin_a[:]], outs=[agout_a[:]])
            edge_phase(0, t0lo, t0hi, adloc0, NHID, T0W, 1, post0,
                       hook=b_hook)

        # ---------------- phase C: AllGather (fp8) ------------------------
        sregs = None
        if "C" in phases:
            if "B" not in phases:
                nc.gpsimd.collective_compute(
                    "AllGather", mybir.AluOpType.bypass,
                    replica_groups=[list(range(NCORES))],
                    ins=[agin_a[:]], outs=[agout_a[:]])
            nc.gpsimd.collective_compute(
                "AllGather", mybir.AluOpType.bypass,
                replica_groups=[list(range(NCORES))],
                ins=[agin_b[:]], outs=[agout_b[:]])
            pid = nc.partition_id(engines=[mybir.EngineType.SP])
            sregs = [nc.sync.snap(((j + pid) % NCORES) * 128)
                     for j in range(NCORES)]

        # ---------------- phase D: layer-1 tables -------------------------
        if "D" in phases:
            DB = 8
            ngt = (N + 127) // 128   # 391
            with ExitStack() as pd:
                xp1 = pd.enter_context(tc.tile_pool(name="pd_x", bufs=4))
                pp1 = pd.enter_context(
                    tc.tile_pool(name="pd_ps", bufs=2, space="PSUM"))
                rp1 = pd.enter_context(tc.tile_pool(name="pd_row", bufs=3))
                for a in range(0, ngt, DB):
                    nsub = min(DB, ngt - a)
                    r0, r1 = a * 128, min((a + DB) * 128, N)
                    hx = xp1.tile([128, DB, 128], F8, tag="hx")
                    hxf = hx[:].rearrange("p g j -> p (g j)")
                    w0 = 0
                    r = r0
                    while r < r1:
                        j = r // NLOC
                        off = r - j * NLOC
                        if off < 3072:
                            end = min(r1, j * NLOC + 3072)
                            src, o2 = agout_a, off
                        else:
                            end = min(r1, (j + 1) * NLOC)
                            src, o2 = agout_b, off - 3072
                        seg = end - r
                        nc.sync.dma_start(
                            out=hxf[:, w0:w0 + seg],
                            in_=src[bass.ds(sregs[j % NCORES], 128),
                                    o2:o2 + seg])
                        w0 += seg
                        r += seg
                    if w0 < DB * 128:
                        nc.vector.memset(hxf[:, w0:DB * 128], 0)
                    hxc = xp1.tile([128, DB, 128], F16, tag="hxc")
                    nc.scalar.copy(hxc[:, 0:nsub, :], hx[:, 0:nsub, :])
                    row = rp1.tile([128, DB, 520], F16, tag="row")
                    for g2 in range(nsub):
                        ps = pp1.tile([128, 512], F32, tag=f"ps{g2 % 2}")
                        nc.tensor.matmul(ps[:], hxc[:, g2, :], W1s[:],
                                         start=True, stop=True)
                        psa = pp1.tile([128, 16], F32, tag=f"psa{g2 % 2}")
                        nc.tensor.matmul(psa[:], hxc[:, g2, :], W1as[:],
                                         start=True, stop=True)
                        e1, e2 = ((nc.scalar, nc.vector) if g2 % 2
                                  else (nc.vector, nc.scalar))
                        e1.copy(row[:, g2, 0:256], ps[:, 0:256]) \
                            if e1 is nc.scalar else \
                            e1.tensor_copy(row[:, g2, 0:256], ps[:, 0:256])
                        if e2 is nc.scalar:
                            e2.copy(row[:, g2, 256:512], ps[:, 256:512])
                        else:
                            e2.tensor_copy(row[:, g2, 256:512],
                                           ps[:, 256:512])
                        nc.vector.tensor_copy(row[:, g2, 512:520],
                                              psa[:, 0:8])
                        g = a + g2
                        if g < lt:
                            nc.scalar.copy(adloc1[:, g * 8:(g + 1) * 8],
                                           psa[:, 8:16])
                    eng = nc.scalar if (a // DB) % 2 else nc.sync
                    tbl_write(eng, t1lo, t1hi, a * 128,
                              row[:, 0:nsub, :], 520, nsub)

        # ---------------- phase E: layer-1 edges + epilogue ---------------
        if "E" in phases:
            fpool = stk.enter_context(tc.tile_pool(name="fin", bufs=1))
            zbig = fpool.tile([128, lt * NCLASS], F32)
            nmxb = fpool.tile([128, lt], F32)
            seb = fpool.tile([128, lt], F32)

            def post1(t, tt, P1, P2, op, pp2, ag_stage):
                r8 = op.tile([128, 8], F32, tag="r8")
                nc.vector.tensor_scalar_add(r8[:], P2, 1e-16)
                nc.vector.reciprocal(r8[:], r8[:])
                nc.vector.tensor_scalar_mul(r8[:], r8[:], 1.0 / HEADS)
                zw = op.tile([128, 512], F32, tag="zw")
                nc.vector.tensor_tensor(
                    out=zw[:].rearrange("p (d h) -> p d h", h=HEADS),
                    in0=P1.rearrange("p (d h) -> p d h", h=HEADS),
                    in1=r8[:].unsqueeze(1).broadcast_to([128, 64, HEADS]),
                    op=mybir.AluOpType.mult)
                z = zbig[:, t * NCLASS:(t + 1) * NCLASS]
                nc.vector.reduce_sum(
                    z, zw[:].rearrange("p (d h) -> p d h", h=HEADS),
                    axis=mybir.AxisListType.X)
                nc.vector.tensor_tensor(out=z, in0=z, in1=b1s[:],
                                        op=mybir.AluOpType.add)
                nmx = nmxb[:, t:t + 1]
                nc.vector.reduce_max(nmx, z, axis=mybir.AxisListType.X,
                                     negate=True)
                ez = op.tile([128, NCLASS], F32, tag="ez")
                nc.scalar.activation(ez[:], z,
                                     mybir.ActivationFunctionType.Exp,
                                     bias=nmx, accum_out=seb[:, t:t + 1])

            edge_phase(1, t1lo, t1hi, adloc1, 512, T1W, 1, post1)
            # batched log-softmax tail
            nc.scalar.activation(seb[:], seb[:],
                                 mybir.ActivationFunctionType.Ln)
            nc.vector.tensor_tensor(
                out=zbig[:].rearrange("p (t c) -> p t c", c=NCLASS),
                in0=zbig[:].rearrange("p (t c) -> p t c", c=NCLASS),
                in1=nmxb[:].unsqueeze(-1).broadcast_to([128, lt, NCLASS]),
                op=mybir.AluOpType.add)
            nc.vector.tensor_tensor(
                out=zbig[:].rearrange("p (t c) -> p t c", c=NCLASS),
                in0=zbig[:].rearrange("p (t c) -> p t c", c=NCLASS),
                in1=seb[:].unsqueeze(-1).broadcast_to([128, lt, NCLASS]),
                op=mybir.AluOpType.subtract)
            nfull = (lt - 1) * 128
            rlast = LAST_ROWS if lt == LT else 128
            nc.sync.dma_start(
                out=out[0:nfull, :].rearrange("(t p) c -> p t c", p=128),
                in_=zbig[:].rearrange("p (t c) -> p t c", c=NCLASS)
                [:, 0:lt - 1, :])
            nc.sync.dma_start(
                out=out[nfull:nfull + rlast, :],
                in_=zbig[0:rlast, (lt - 1) * NCLASS:lt * NCLASS])

    nc.compile()
    return nc


# --------------------------------------------------------------------------
# entry point
# --------------------------------------------------------------------------

def kernel(**inputs) -> np.ndarray:
    NLk, NHk, in_maps = _prep_inputs(**inputs)
    key = (NLk, NHk)
    if key not in _cache:
        _cache[key] = build(NLk, NHk)
    nc = _cache[key]
    res = run_bass_kernel_spmd(nc, in_maps, list(range(NCORES)))
    return np.concatenate([res.results[c]["out"] for c in range(NCORES)], 0)


# revision 32
# speedup vs baseline: 1.0156x; 1.0156x over previous
"""2-layer GAT (nn_GAT_31490700214331) on 8 Trainium2 NeuronCores.

Strategy (dst-sharded, SPMD, per-core-rotated node layout), v2:
  - Nodes block-partitioned: core c owns nodes [c*6250, (c+1)*6250); every
    table on core c uses a rotated row order so local nodes are rows 0..6249
    and the SPMD program needs no core-dependent offsets.
  - Layer tables (t0: [h0|as0] f16 512B-pitch rows, t1: [h1|as1] f16
    1280B-pitch rows) are computed replicated on every core; per-dst-tile
    edges fetch source rows with dma_gather (int16 indices, lo/hi table
    split keeps indices < 32768).
  - alpha_dst is NOT gathered per edge: each core stashes its local tiles'
    ad columns in SBUF (adloc), builds a transposed incidence one-hot incT
    from a host-provided transposed dst-id row (PE ones-broadcast + DVE
    is_equal), and reads ad per edge-slot with tiny [128x128]@[128x8]
    matmuls. Denominators ride as 8 extra columns of the aggregation matmul.
  - Edge softmax is safe without segment-max (|e| small); aggregation is
    per-chunk incidence matmuls accumulating in PSUM.
  - Between layers the ELU'd hidden state is AllGather'd in fp8
    (6.25 MB instead of 12.5 MB), rotated into per-core order with
    partition-id-offset DMA reads in phase D.
  - alpha projections fold into the weight matmuls on the host:
    W0a=[256,16] / W1a=[128,16] give as/ad as extra psum columns.

Self-contained: call kernel(**inputs) with the full-problem arrays.
"""
import numpy as np
from contextlib import ExitStack

import concourse.bacc as bacc
import concourse.bass as bass
import concourse.mybir as mybir
from concourse.tile import TileContext
from concourse.bass_utils import run_bass_kernel_spmd

F16 = mybir.dt.float16
F32 = mybir.dt.float32
F8 = mybir.dt.float8e4
I16 = mybir.dt.int16

N = 50000
NFEAT = 256
NHID = 128
NCLASS = 64
HEADS = 8
SLOPE = 0.2
NCORES = 8
NLOC = N // NCORES           # 6250
LT = (NLOC + 127) // 128     # 49 local dst tiles
LAST_ROWS = NLOC - (LT - 1) * 128   # 106 rows in the last tile
GT = 392                     # global node tiles (392*128 = 50176)
GROWS = GT * 128
SPLIT = 25088                # low/high gather-table split (196 tiles)
SENT = 300.0                 # dst_rel sentinel for padding slots
T0W = 256                    # t0 row pitch (f16): [h0(128)|as0(8)|junk]
T1W = 640                    # t1 row pitch (f16): [h1(512)|as1(8)|junk]

_cache = {}


# --------------------------------------------------------------------------
# host-side preparation
# --------------------------------------------------------------------------

def _wrap_idx(idx):
    """[n] int -> [128, n//16] int16 wrapped gather-index layout."""
    n = idx.shape[0]
    assert n % 16 == 0
    w = idx.reshape(n // 16, 16).T.astype(np.int16)
    return np.tile(w, (8, 1))


def _prep_edges(src, dst):
    cores = []
    for c in range(NCORES):
        m = (dst >= c * NLOC) & (dst < (c + 1) * NLOC)
        s = src[m].astype(np.int64)
        d = dst[m].astype(np.int64) - c * NLOC
        order = np.argsort(d, kind="stable")
        s, d = s[order], d[order]
        s_rot = (s - c * NLOC) % N
        tiles = []
        for t in range(LT):
            sel = (d >= t * 128) & (d < (t + 1) * 128)
            st, dt = s_rot[sel], d[sel] - t * 128
            lo = st < SPLIT
            tiles.append((st[lo], dt[lo], st[~lo] - SPLIT, dt[~lo]))
        cores.append(tiles)
    nl = max(len(t[0]) for tl in cores for t in tl)
    nh = max(len(t[2]) for tl in cores for t in tl)
    NL = max(1, (nl + 127) // 128)
    NH = max(1, (nh + 127) // 128)
    assert NL * 128 <= 1024 and NH * 128 <= 1024, (NL, NH)
    CH = NL + NH

    out = []
    for c in range(NCORES):
        eil = np.zeros((LT, 128, NL * 8), np.int16)
        eih = np.zeros((LT, 128, NH * 8), np.int16)
        edr = np.full((LT, 128, CH), SENT, np.float16)
        edrT = np.full((LT, CH * 128), SENT, np.float16)
        for t in range(LT):
            sl, dl, sh, dh = cores[c][t]
            il = np.zeros(NL * 128, np.int64)
            il[: len(sl)] = sl
            ih = np.zeros(NH * 128, np.int64)
            ih[: len(sh)] = sh
            eil[t] = _wrap_idx(il)
            eih[t] = _wrap_idx(ih)
            rl = np.full(NL * 128, SENT)
            rl[: len(dl)] = dl
            rh = np.full(NH * 128, SENT)
            rh[: len(dh)] = dh
            flat = np.concatenate([rl, rh])
            edr[t] = flat.reshape(CH, 128).T.astype(np.float16)
            edrT[t] = flat.astype(np.float16)
        out.append(dict(eil=np.ascontiguousarray(eil),
                        eih=np.ascontiguousarray(eih),
                        edr=np.ascontiguousarray(edr),
                        edrT=np.ascontiguousarray(edrT)))
    return NL, NH, out


def _prep_inputs(x, edge_index, W0, a_src0, a_dst0, b0, W1, a_src1, a_dst1,
                 b1):
    src = np.asarray(edge_index[0]).astype(np.int64)
    dst = np.asarray(edge_index[1]).astype(np.int64)
    NL, NH, edata = _prep_edges(src, dst)
    CH = NL + NH

    def bd(a):  # [H, D] -> blockdiag [H*D, H]
        a = np.asarray(a, np.float32)
        H, D = a.shape
        m = np.zeros((H * D, H), np.float32)
        for h in range(H):
            m[h * D:(h + 1) * D, h] = a[h]
        return m

    W0 = np.asarray(W0, np.float32)
    W1 = np.asarray(W1, np.float32)
    W0a = np.concatenate([W0 @ bd(a_src0), W0 @ bd(a_dst0)], 1)  # [256, 16]
    # head-innermost feature interleave: new col d*8+h <- old col h*D+d
    perm0 = np.array([(f % 8) * 16 + f // 8 for f in range(128)])
    perm1 = np.array([(f % 8) * 64 + f // 8 for f in range(512)])
    W0cat = np.concatenate([W0[:, perm0], W0a], 1)               # [256, 144]
    W1a = np.concatenate([W1 @ bd(a_src1), W1 @ bd(a_dst1)], 1)  # [128, 16]

    x = np.asarray(x, np.float32)
    ident = np.eye(128, dtype=np.float16)
    # colio_rep[p, j*CH+c] = j ; rowio[p, :] = p
    colio_rep = np.repeat(np.arange(128, dtype=np.float16), CH)[None, :]
    colio_rep = np.ascontiguousarray(np.tile(colio_rep, (128, 1)))
    rowio = np.tile(np.arange(128, dtype=np.float16)[:, None], (1, CH * 128))
    b0b = np.tile(np.asarray(b0, np.float32)[None, :], (128, 1))
    b1b = np.tile(np.asarray(b1, np.float32)[None, :], (128, 1))

    in_maps = []
    for c in range(NCORES):
        rot = np.roll(np.arange(N), -c * NLOC)
        xr = np.zeros((GROWS, NFEAT), np.float16)
        xr[:N] = x[rot].astype(np.float16)
        xtt = xr.reshape(GROWS // 128, 128, 2, 128).transpose(0, 3, 2, 1)
        m = dict(
            xT=np.ascontiguousarray(xtt),
            W0=np.ascontiguousarray(
                W0cat.astype(np.float16).reshape(2, 128, NHID + 16)),
            W1=np.ascontiguousarray(W1[perm0][:, perm1].astype(np.float16)),
            W1a=np.ascontiguousarray(W1a[perm0].astype(np.float16)),
            b0b=np.ascontiguousarray(b0b[:, perm0]), b1b=b1b,
            ident=ident, colio_rep=colio_rep, rowio=np.ascontiguousarray(rowio),
            **edata[c],
        )
        in_maps.append(m)
    return NL, NH, in_maps


# --------------------------------------------------------------------------
# device program
# --------------------------------------------------------------------------

def build(NL, NH, lt=LT, gt=GT, phases="ABCDE"):
    CH = NL + NH
    NLI = NL * 128
    NHI = NH * 128

    nc = bacc.Bacc("TRN2")
    xT = nc.dram_tensor("xT", [GROWS // 128, 128, 2, 128], F16,
                        kind="ExternalInput")
    W0i = nc.dram_tensor("W0", [2, 128, NHID + 16], F16,
                         kind="ExternalInput")
    W1i = nc.dram_tensor("W1", [NHID, 512], F16, kind="ExternalInput")
    W1ai = nc.dram_tensor("W1a", [NHID, 16], F16, kind="ExternalInput")
    b0bi = nc.dram_tensor("b0b", [128, NHID], F32, kind="ExternalInput")
    b1bi = nc.dram_tensor("b1b", [128, NCLASS], F32, kind="ExternalInput")
    identi = nc.dram_tensor("ident", [128, 128], F16, kind="ExternalInput")
    colrepi = nc.dram_tensor("colio_rep", [128, 128 * CH], F16,
                             kind="ExternalInput")
    rowioi = nc.dram_tensor("rowio", [128, CH * 128], F16,
                            kind="ExternalInput")
    eili = nc.dram_tensor("eil", [lt, 128, NL * 8], I16,
                          kind="ExternalInput")
    eihi = nc.dram_tensor("eih", [lt, 128, NH * 8], I16,
                          kind="ExternalInput")
    edri = nc.dram_tensor("edr", [lt, 128, CH], F16, kind="ExternalInput")
    edrTi = nc.dram_tensor("edrT", [lt, CH * 128], F16,
                           kind="ExternalInput")
    out = nc.dram_tensor("out", [NLOC, NCLASS], F32, kind="ExternalOutput")

    with TileContext(nc) as tc, ExitStack() as stk:
        rg = {}
        for bs in {4, 2, 1, lt % 4 or 4, lt % 2 or 2}:
            rg[("lo", bs)] = nc.gpsimd.to_reg(bs * NLI)
            rg[("hi", bs)] = nc.gpsimd.to_reg(bs * NHI)
        dpool = stk.enter_context(
            tc.tile_pool(name="dram", bufs=1, space="DRAM"))
        t0lo = dpool.tile([SPLIT, T0W], F16, tag="t0lo")
        t0hi = dpool.tile([GROWS - SPLIT, T0W], F16, tag="t0hi")
        t1lo = dpool.tile([SPLIT, T1W], F16, tag="t1lo")
        t1hi = dpool.tile([GROWS - SPLIT, T1W], F16, tag="t1hi")
        CSP = 3072   # collective split column (end of B load-batch 2)
        agin_a = dpool.tile([128, CSP], F8, tag="agin_a")
        agin_b = dpool.tile([128, NLOC - CSP], F8, tag="agin_b")
        agout_a = dpool.tile([NCORES * 128, CSP], F8, tag="agout_a",
                             addr_space="Shared")
        agout_b = dpool.tile([NCORES * 128, NLOC - CSP], F8, tag="agout_b",
                             addr_space="Shared")

        cpool = stk.enter_context(tc.tile_pool(name="const", bufs=1))
        W0s = cpool.tile([128, 2, NHID + 16], F16)
        nc.sync.dma_start(out=W0s[:], in_=W0i.rearrange("k p n -> p k n"))
        W1s = cpool.tile([128, 512], F16)
        nc.sync.dma_start(out=W1s[:], in_=W1i[:])
        W1as = cpool.tile([128, 16], F16)
        nc.sync.dma_start(out=W1as[:], in_=W1ai[:])
        b0s = cpool.tile([128, NHID], F32)
        nc.sync.dma_start(out=b0s[:], in_=b0bi[:])
        b1s = cpool.tile([128, NCLASS], F32)
        nc.sync.dma_start(out=b1s[:], in_=b1bi[:])
        idents = cpool.tile([128, 128], F16)
        nc.sync.dma_start(out=idents[:], in_=identi[:])
        colreps = cpool.tile([128, 128, CH], F16)
        nc.sync.dma_start(out=colreps[:],
                          in_=colrepi.rearrange("p (j c) -> p j c", c=CH))
        rowios = cpool.tile([128, CH * 128], F16)
        nc.sync.dma_start(out=rowios[:], in_=rowioi[:])
        ones1 = cpool.tile([1, 128], F16)
        nc.vector.memset(ones1[:], 1.0)
        adloc0 = cpool.tile([128, lt * 8], F16)
        adloc1 = cpool.tile([128, lt * 8], F16)

        def tbl_write(eng, tlo, thi, g0, row_ap, w, nt):
            """write nt row-tiles (128 rows each) starting at table row g0"""
            rows = nt * 128
            if g0 + rows <= SPLIT:
                eng.dma_start(
                    out=tlo[g0:g0 + rows, 0:w]
                    .rearrange("(g p) w -> p g w", p=128),
                    in_=row_ap)
            elif g0 >= SPLIT:
                o = g0 - SPLIT
                eng.dma_start(
                    out=thi[o:o + rows, 0:w]
                    .rearrange("(g p) w -> p g w", p=128),
                    in_=row_ap)
            else:
                k = (SPLIT - g0) // 128
                eng.dma_start(
                    out=tlo[g0:SPLIT, 0:w]
                    .rearrange("(g p) w -> p g w", p=128),
                    in_=row_ap[:, 0:k, :])
                eng.dma_start(
                    out=thi[0:rows - k * 128, 0:w]
                    .rearrange("(g p) w -> p g w", p=128),
                    in_=row_ap[:, k:nt, :])

        # ---------------- phase A: layer-0 tables (replicated) ------------
        if "A" in phases:
            AB = 14
            with ExitStack() as pa:
                xp = pa.enter_context(tc.tile_pool(name="pa_x", bufs=3))
                pp = pa.enter_context(
                    tc.tile_pool(name="pa_ps", bufs=1, space="PSUM"))
                rp = pa.enter_context(tc.tile_pool(name="pa_row", bufs=3))
                assert gt % AB == 0
                for i in range(gt // AB):
                    xa = xp.tile([128, AB, 2, 128], F16, tag="xa")
                    nc.sync.dma_start(
                        out=xa[:],
                        in_=xT[AB * i:AB * (i + 1)].rearrange(
                            "g p k j -> p g k j"))
                    row = rp.tile([128, AB, 136], F16, tag="row")
                    for g2 in range(AB):
                        ps = pp.tile([128, NHID + 16], F32, tag=f"ps{g2 % 4}")
                        nc.tensor.matmul(ps[:], xa[:, g2, 0, :], W0s[:, 0, :],
                                         start=True, stop=False)
                        nc.tensor.matmul(ps[:], xa[:, g2, 1, :], W0s[:, 1, :],
                                         start=False, stop=True)
                        nc.vector.tensor_copy(row[:, g2, :], ps[:, 0:136])
                        g = AB * i + g2
                        if g < lt:
                            nc.scalar.copy(adloc0[:, g * 8:(g + 1) * 8],
                                           ps[:, 136:144])
                    eng = nc.scalar if i % 2 else nc.sync
                    tbl_write(eng, t0lo, t0hi, i * AB * 128, row[:], 136, AB)

        # ---------------- shared edge phase -------------------------------
        def edge_phase(layer, tlo, thi, adloc, fdim, trow, GB,
                       post_fn, hook=None):
            D = fdim // HEADS
            LB = 8   # load batch (tiles)
            with ExitStack() as pb:
                ii = pb.enter_context(tc.tile_pool(name=f"ix{layer}", bufs=3))
                i2 = pb.enter_context(tc.tile_pool(name=f"i2{layer}", bufs=1))
                gp = pb.enter_context(tc.tile_pool(
                    name=f"gg{layer}", bufs=3 if layer else 4))
                rp2 = pb.enter_context(
                    tc.tile_pool(name=f"rh{layer}", bufs=3))
                rp3 = pb.enter_context(
                    tc.tile_pool(name=f"rr{layer}", bufs=3 if layer else 4))
                pp2 = pb.enter_context(
                    tc.tile_pool(name=f"ps{layer}", bufs=2, space="PSUM"))
                pp1b = pb.enter_context(
                    tc.tile_pool(name=f"p1{layer}", bufs=1, space="PSUM"))
                op = pb.enter_context(tc.tile_pool(name=f"po{layer}", bufs=4))

                for l0 in range(0, lt, LB):
                    lb = min(LB, lt - l0)
                    il_s = ii.tile([128, LB, NL * 8], I16, tag="il")
                    nc.sync.dma_start(
                        out=il_s[:, 0:lb, :],
                        in_=eili[l0:l0 + lb].rearrange("t p w -> p t w"))
                    ih_s = ii.tile([128, LB, NH * 8], I16, tag="ih")
                    nc.sync.dma_start(
                        out=ih_s[:, 0:lb, :],
                        in_=eihi[l0:l0 + lb].rearrange("t p w -> p t w"))
                    drs = ii.tile([128, LB, CH], F16, tag="dr")
                    nc.scalar.dma_start(
                        out=drs[:, 0:lb, :],
                        in_=edri[l0:l0 + lb].rearrange("t p w -> p t w"))
                    ag_stage = None
                    if layer == 0:
                        ag_stage = op.tile([128, LB, 128], F8, tag="ag")

                    drts = i2.tile([1, LB, CH * 128], F16, tag="drt")
                    nc.scalar.dma_start(
                        out=drts[0:1, 0:lb, :], in_=edrTi[l0:l0 + lb])
                    for s0 in range(0, lb, GB):
                        gb = min(GB, lb - s0)
                        Glo = gp.tile([128, GB * NL, trow], F16, tag="glo")
                        nc.gpsimd.dma_gather(
                            Glo[:, 0:gb * NL, :], tlo[:],
                            il_s[:, s0:s0 + gb, :]
                            .rearrange("p t w -> p (t w)"),
                            gb * NLI, rg[("lo", gb)], trow)
                        Ghi = gp.tile([128, GB * NH, trow], F16, tag="ghi")
                        nc.gpsimd.dma_gather(
                            Ghi[:, 0:gb * NH, :], thi[:],
                            ih_s[:, s0:s0 + gb, :]
                            .rearrange("p t w -> p (t w)"),
                            gb * NHI, rg[("hi", gb)], trow)

                        for u in range(gb):
                            tt = s0 + u           # index within load batch
                            t = l0 + tt           # tile index
                            gl = Glo[:, u * NL:(u + 1) * NL, :]
                            gh = Ghi[:, u * NH:(u + 1) * NH, :]

                            # transposed incidence one-hot via ones-broadcast
                            drB16 = rp2.tile([128, CH * 128], F16, tag="drB")
                            quart = (CH * 128) // 4
                            assert quart <= 512
                            for hh in range(4):
                                drB = pp1b.tile([128, quart], F32,
                                                tag=f"drB{hh % 2}")
                                nc.tensor.matmul(
                                    drB[:], ones1[:],
                                    drts[0:1, tt,
                                         hh * quart:(hh + 1) * quart],
                                    start=True, stop=True)
                                nc.scalar.copy(
                                    drB16[:, hh * quart:(hh + 1) * quart],
                                    drB[:])
                            incT = rp2.tile([128, CH * 128], F16, tag="incT")
                            nc.vector.tensor_tensor(
                                out=incT[:], in0=drB16[:], in1=rowios[:],
                                op=mybir.AluOpType.is_equal)
                            # ad per edge slot: [K=128d, M=128slot]^T @ [K,8]
                            adsp = pp1b.tile([128, CH, 8], F32,
                                             tag=f"adsp{t % 2}")
                            for ch in range(CH):
                                nc.tensor.matmul(
                                    adsp[:, ch, :],
                                    incT[:, ch * 128:(ch + 1) * 128],
                                    adloc[:, t * 8:(t + 1) * 8],
                                    start=True, stop=True)

                            # incidence (j-major for 2x DVE)
                            inc = rp3.tile([128, 128, CH], F16, tag="inc")
                            nc.vector.tensor_tensor(
                                out=inc[:],
                                in0=drs[:, tt, :].unsqueeze(1)
                                .broadcast_to([128, 128, CH]),
                                in1=colreps[:],
                                op=mybir.AluOpType.is_equal)

                            # EX = exp(prelu(as + ad)) into R[:, :, fdim:]
                            R = rp3.tile([128, CH, fdim + 8], F16, tag="R")
                            nc.vector.tensor_tensor(
                                out=R[:, 0:NL, fdim:fdim + 8],
                                in0=gl[:, :, fdim:fdim + 8],
                                in1=adsp[:, 0:NL, :],
                                op=mybir.AluOpType.add)
                            nc.vector.tensor_tensor(
                                out=R[:, NL:CH, fdim:fdim + 8],
                                in0=gh[:, :, fdim:fdim + 8],
                                in1=adsp[:, NL:CH, :],
                                op=mybir.AluOpType.add)
                            ex = R[:, :, fdim:fdim + 8]
                            # prelu(e) = e + (1-slope)*relu(-e)
                            exn = rp3.tile([128, CH, 8], F16, tag="exn")
                            nc.scalar.activation(
                                exn[:], ex,
                                mybir.ActivationFunctionType.Relu,
                                scale=-(1.0 - SLOPE))
                            nc.vector.tensor_tensor(
                                out=ex, in0=ex, in1=exn[:],
                                op=mybir.AluOpType.add)
                            nc.scalar.activation(
                                ex, ex, mybir.ActivationFunctionType.Exp)

                            # R = h * EX (head-innermost broadcast)
                            nc.vector.tensor_tensor(
                                out=R[:, 0:NL, 0:fdim]
                                .rearrange("p c (d h) -> p c d h", h=HEADS),
                                in0=gl[:, :, 0:fdim]
                                .rearrange("p c (d h) -> p c d h", h=HEADS),
                                in1=R[:, 0:NL, fdim:fdim + 8].unsqueeze(2)
                                .broadcast_to([128, NL, D, HEADS]),
                                op=mybir.AluOpType.mult)
                            nc.vector.tensor_tensor(
                                out=R[:, NL:CH, 0:fdim]
                                .rearrange("p c (d h) -> p c d h", h=HEADS),
                                in0=gh[:, :, 0:fdim]
                                .rearrange("p c (d h) -> p c d h", h=HEADS),
                                in1=R[:, NL:CH, fdim:fdim + 8].unsqueeze(2)
                                .broadcast_to([128, NH, D, HEADS]),
                                op=mybir.AluOpType.mult)

                            # aggregate (+denominators merged when they fit
                            # a single PSUM bank)
                            if fdim + 8 <= 512:
                                P = pp2.tile([128, fdim + 8], F32, tag="P1")
                                for ch in range(CH):
                                    nc.tensor.matmul(
                                        P[:], inc[:, :, ch], R[:, ch, :],
                                        start=(ch == 0), stop=(ch == CH - 1))
                                P1, P2 = P[:, 0:fdim], P[:, fdim:fdim + 8]
                            else:
                                P1t = pp2.tile([128, fdim], F32, tag="P1")
                                for ch in range(CH):
                                    nc.tensor.matmul(
                                        P1t[:], inc[:, :, ch],
                                        R[:, ch, 0:fdim],
                                        start=(ch == 0), stop=(ch == CH - 1))
                                P2t = pp2.tile([128, 8], F32, tag="P2")
                                for ch in range(CH):
                                    nc.tensor.matmul(
                                        P2t[:], inc[:, :, ch],
                                        R[:, ch, fdim:fdim + 8],
                                        start=(ch == 0), stop=(ch == CH - 1))
                                P1, P2 = P1t[:], P2t[:]
                            post_fn(t, tt, P1, P2, op, pp2, ag_stage)

                    if layer == 0:
                        c0 = l0 * 128
                        cn = min(lb * 128, NLOC - c0)
                        if c0 < 3072:
                            dst = agin_a[:, c0:c0 + cn]
                        else:
                            dst = agin_b[:, c0 - 3072:c0 - 3072 + cn]
                        nc.scalar.dma_start(
                            out=dst,
                            in_=ag_stage[:]
                            .rearrange("p t j -> p (t j)")[:, 0:cn])
                    if hook is not None:
                        hook(l0)

        # ---- L0 post: softmax-div, +b0, ELU, transpose, stage fp8 --------
        def post0(t, tt, P1, P2, op, pp2, ag_stage):
            r8 = op.tile([128, 8], F32, tag="r8")
            nc.vector.tensor_scalar_add(r8[:], P2, 1e-16)
            nc.vector.reciprocal(r8[:], r8[:])
            z = op.tile([128, NHID], F32, tag="z")
            nc.vector.tensor_tensor(
                out=z[:].rearrange("p (d h) -> p d h", h=HEADS),
                in0=P1.rearrange("p (d h) -> p d h", h=HEADS),
                in1=r8[:].unsqueeze(1).broadcast_to([128, 16, HEADS]),
                op=mybir.AluOpType.mult)
            nc.vector.tensor_tensor(out=z[:], in0=z[:], in1=b0s[:],
                                    op=mybir.AluOpType.add)
            zm = op.tile([128, NHID], F16, tag="zm")
            nc.scalar.activation(zm[:], z[:],
                                 mybir.ActivationFunctionType.Relu,
                                 scale=-1.0)
            nc.scalar.activation(zm[:], zm[:],
                                 mybir.ActivationFunctionType.Exp,
                                 scale=-1.0)
            zp = op.tile([128, NHID], F16, tag="zp")
            nc.scalar.activation(zp[:], z[:],
                                 mybir.ActivationFunctionType.Relu)
            h1 = op.tile([128, NHID], F16, tag="h1")
            nc.vector.tensor_tensor(out=h1[:], in0=zp[:], in1=zm[:],
                                    op=mybir.AluOpType.add)
            nc.vector.tensor_scalar_add(h1[:], h1[:], -1.0)
            pst = pp2.tile([128, 128], F16, tag="pst")
            nc.tensor.transpose(pst[:], h1[:], idents[:])
            nc.vector.tensor_copy(ag_stage[:, tt, :], pst[:])

        if "B" in phases:
            def b_hook(l0):
                if l0 == 16 and "C" in phases:
                    nc.gpsimd.collective_compute(
                        "AllGather", mybir.AluOpType.bypass,
                        replica_groups=[list(range(NCORES))],
                        ins=[agin_a[:]], outs=[agout_a[:]])
            edge_phase(0, t0lo, t0hi, adloc0, NHID, T0W, 1, post0,
                       hook=b_hook)

        # ---------------- phase C: AllGather (fp8) ------------------------
        sregs = None
        if "C" in phases:
            if "B" not in phases:
                nc.gpsimd.collective_compute(
                    "AllGather", mybir.AluOpType.bypass,
                    replica_groups=[list(range(NCORES))],
                    ins=[agin_a[:]], outs=[agout_a[:]])
            nc.gpsimd.collective_compute(
                "AllGather", mybir.AluOpType.bypass,
                replica_groups=[list(range(NCORES))],
                ins=[agin_b[:]], outs=[agout_b[:]])
            pid = nc.partition_id(engines=[mybir.EngineType.SP])
            sregs = [nc.sync.snap(((j + pid) % NCORES) * 128)
                     for j in range(NCORES)]

        # ---------------- phase D: layer-1 tables -------------------------
        if "D" in phases:
            DB = 8
            ngt = (N + 127) // 128   # 391
            with ExitStack() as pd:
                xp1 = pd.enter_context(tc.tile_pool(name="pd_x", bufs=4))
                pp1 = pd.enter_context(
                    tc.tile_pool(name="pd_ps", bufs=2, space="PSUM"))
                rp1 = pd.enter_context(tc.tile_pool(name="pd_row", bufs=3))
                for a in range(0, ngt, DB):
                    nsub = min(DB, ngt - a)
                    r0, r1 = a * 128, min((a + DB) * 128, N)
                    hx = xp1.tile([128, DB, 128], F8, tag="hx")
                    hxf = hx[:].rearrange("p g j -> p (g j)")
                    w0 = 0
                    r = r0
                    while r < r1:
                        j = r // NLOC
                        off = r - j * NLOC
                        if off < 3072:
                            end = min(r1, j * NLOC + 3072)
                            src, o2 = agout_a, off
                        else:
                            end = min(r1, (j + 1) * NLOC)
                            src, o2 = agout_b, off - 3072
                        seg = end - r
                        nc.sync.dma_start(
                            out=hxf[:, w0:w0 + seg],
                            in_=src[bass.ds(sregs[j % NCORES], 128),
                                    o2:o2 + seg])
                        w0 += seg
                        r += seg
                    if w0 < DB * 128:
                        nc.vector.memset(hxf[:, w0:DB * 128], 0)
                    hxc = xp1.tile([128, DB, 128], F16, tag="hxc")
                    hcut = max(1, nsub // 2)
                    nc.scalar.copy(hxc[:, 0:hcut, :], hx[:, 0:hcut, :])
                    nc.vector.tensor_copy(hxc[:, hcut:nsub, :],
                                          hx[:, hcut:nsub, :])
                    row = rp1.tile([128, DB, 520], F16, tag="row")
                    for g2 in range(nsub):
                        ps = pp1.tile([128, 512], F32, tag=f"ps{g2 % 2}")
                        nc.tensor.matmul(ps[:], hxc[:, g2, :], W1s[:],
                                         start=True, stop=True)
                        psa = pp1.tile([128, 16], F32, tag=f"psa{g2 % 2}")
                        nc.tensor.matmul(psa[:], hxc[:, g2, :], W1as[:],
                                         start=True, stop=True)
                        e1, e2 = ((nc.scalar, nc.vector) if g2 % 2
                                  else (nc.vector, nc.scalar))
                        e1.copy(row[:, g2, 0:256], ps[:, 0:256]) \
                            if e1 is nc.scalar else \
                            e1.tensor_copy(row[:, g2, 0:256], ps[:, 0:256])
                        if e2 is nc.scalar:
                            e2.copy(row[:, g2, 256:512], ps[:, 256:512])
                        else:
                            e2.tensor_copy(row[:, g2, 256:512],
                                           ps[:, 256:512])
                        nc.vector.tensor_copy(row[:, g2, 512:520],
                                              psa[:, 0:8])
                        g = a + g2
                        if g < lt:
                            nc.scalar.copy(adloc1[:, g * 8:(g + 1) * 8],
                                           psa[:, 8:16])
                    eng = nc.scalar if (a // DB) % 2 else nc.sync
                    tbl_write(eng, t1lo, t1hi, a * 128,
                              row[:, 0:nsub, :], 520, nsub)

        # ---------------- phase E: layer-1 edges + epilogue ---------------
        if "E" in phases:
            fpool = stk.enter_context(tc.tile_pool(name="fin", bufs=1))
            zbig = fpool.tile([128, lt * NCLASS], F32)
            nmxb = fpool.tile([128, lt], F32)
            seb = fpool.tile([128, lt], F32)

            def post1(t, tt, P1, P2, op, pp2, ag_stage):
                r8 = op.tile([128, 8], F32, tag="r8")
                nc.vector.tensor_scalar_add(r8[:], P2, 1e-16)
                nc.vector.reciprocal(r8[:], r8[:])
                nc.vector.tensor_scalar_mul(r8[:], r8[:], 1.0 / HEADS)
                zw = op.tile([128, 512], F32, tag="zw")
                nc.vector.tensor_tensor(
                    out=zw[:].rearrange("p (d h) -> p d h", h=HEADS),
                    in0=P1.rearrange("p (d h) -> p d h", h=HEADS),
                    in1=r8[:].unsqueeze(1).broadcast_to([128, 64, HEADS]),
                    op=mybir.AluOpType.mult)
                z = zbig[:, t * NCLASS:(t + 1) * NCLASS]
                nc.vector.reduce_sum(
                    z, zw[:].rearrange("p (d h) -> p d h", h=HEADS),
                    axis=mybir.AxisListType.X)
                nc.vector.tensor_tensor(out=z, in0=z, in1=b1s[:],
                                        op=mybir.AluOpType.add)
                nmx = nmxb[:, t:t + 1]
                nc.vector.reduce_max(nmx, z, axis=mybir.AxisListType.X,
                                     negate=True)
                ez = op.tile([128, NCLASS], F32, tag="ez")
                nc.scalar.activation(ez[:], z,
                                     mybir.ActivationFunctionType.Exp,
                                     bias=nmx, accum_out=seb[:, t:t + 1])

            edge_phase(1, t1lo, t1hi, adloc1, 512, T1W, 1, post1)
            # batched log-softmax tail
            nc.scalar.activation(seb[:], seb[:],
                                 mybir.ActivationFunctionType.Ln)
            nc.vector.tensor_tensor(
                out=zbig[:].rearrange("p (t c) -> p t c", c=NCLASS),
                in0=zbig[:].rearrange("p (t c) -> p t c", c=NCLASS),
                in1=nmxb[:].unsqueeze(-1).broadcast_to([128, lt, NCLASS]),
                op=mybir.AluOpType.add)
            nc.vector.tensor_tensor(
                out=zbig[:].rearrange("p (t c) -> p t c", c=NCLASS),
                in0=zbig[:].rearrange("p (t c) -> p t c", c=NCLASS),
                in1=seb[:].unsqueeze(-1).broadcast_to([128, lt, NCLASS]),
                op=mybir.AluOpType.subtract)
            nfull = (lt - 1) * 128
            rlast = LAST_ROWS if lt == LT else 128
            nc.sync.dma_start(
                out=out[0:nfull, :].rearrange("(t p) c -> p t c", p=128),
                in_=zbig[:].rearrange("p (t c) -> p t c", c=NCLASS)
                [:, 0:lt - 1, :])
            nc.sync.dma_start(
                out=out[nfull:nfull + rlast, :],
                in_=zbig[0:rlast, (lt - 1) * NCLASS:lt * NCLASS])

    nc.compile()
    return nc


# --------------------------------------------------------------------------
# entry point
# --------------------------------------------------------------------------

def kernel(**inputs) -> np.ndarray:
    NLk, NHk, in_maps = _prep_inputs(**inputs)
    key = (NLk, NHk)
    if key not in _cache:
        _cache[key] = build(NLk, NHk)
    nc = _cache[key]
    res = run_bass_kernel_spmd(nc, in_maps, list(range(NCORES)))
    return np.concatenate([res.results[c]["out"] for c in range(NCORES)], 0)
